# revision 37
# baseline (speedup 1.0000x reference)
"""VQ codebook kernel (nn_Codebook) for 8 Trainium2 NeuronCores.

Computes, for encode (32,256,32,32) f32 and templat (2048,256) f32:
    dist  = ||e||^2 + ||t||^2 - 2 e.t          (b, m, h, w)
    zsoft = softmax(-dist, axis=m)             (32, 2048, 32, 32) f32
    zidx  = argmax(zsoft, axis=m)              (32, 32, 32) int32

Sharding: data-parallel over batch (4 batches per core), codebook replicated.

Numerical strategy (matters for argmax ties): the reference's dist is
computed in f32 at magnitude ~||e||^2 ~ 256, which quantizes the per-m
variation to a ~1.5e-5 grid. We mimic that arithmetic exactly on device:
  B = fl(fl(e2 + t2) - 2*cross)   with 2*cross from a bf16-split matmul
      (hi*hi + hi*lo + lo*hi, error ~3e-7 << grid)
  zsoft_unnorm = exp(e2 - B)      (ACT, fused bias, exact Sterbenz shift)
  zsoft = zsoft_unnorm * (1/sum)  per pixel
which reproduces the reference's argmax decisions (validated: 0 flips of
32768 vs a CPU-jax reference) while keeping zsoft within ~1e-7 relative.
"""

import sys

if '/opt/trn_rl_repo' not in sys.path:
    sys.path.insert(0, '/opt/trn_rl_repo')

import types
import numpy as np
import ml_dtypes
from contextlib import ExitStack

import concourse.tile as tile
from concourse import bacc, mybir
from concourse.bass_utils import run_bass_kernel_spmd

# bass_utils imports antenv.axon_hooks when tracing is requested (e.g. via a
# stray BASS_TRACE env). This image's antenv lacks that module; register a
# no-op fallback so tracing degrades gracefully instead of ImportError-ing.
if 'antenv.axon_hooks' not in sys.modules:
    _m = types.ModuleType('antenv.axon_hooks')
    _m._hook = None
    _m.set_axon_ntff_profile_hook = lambda h: setattr(_m, '_hook', h)
    _m.get_axon_ntff_profile_hook = lambda: _m._hook
    sys.modules['antenv.axon_hooks'] = _m

B_FULL, C, H, W = 32, 256, 32, 32
M = 2048
HW = H * W
N_CORES = 8
B_CORE = B_FULL // N_CORES          # 4 batches per core
N_TILES = B_CORE * (HW // 128)      # 32 pixel-tiles of 128 per core

F32 = mybir.dt.float32
BF16 = mybir.dt.bfloat16

_NC_CACHE = {}


def build_program(n_tiles=N_TILES, mm_mode='bf16split', cfg=None):
    cfg = dict({'mm_bufs': 6, 'tr_bufs': 2, 'sb_bufs': 2}, **(cfg or {}))
    nc = bacc.Bacc('TRN2', target_bir_lowering=False, debug=False,
                   num_devices=N_CORES)

    F32R = mybir.dt.float32r
    if mm_mode == 'f32r':
        d_ehi = nc.dram_tensor('ehi', [B_CORE, C, HW], F32R, kind='ExternalInput').ap()
        d_thi = nc.dram_tensor('thi', [C, M], F32R, kind='ExternalInput').ap()
        d_elo = d_tlo = None
    else:
        d_ehi = nc.dram_tensor('ehi', [B_CORE, C, HW], BF16, kind='ExternalInput').ap()
        d_elo = nc.dram_tensor('elo', [B_CORE, C, HW], BF16, kind='ExternalInput').ap()
        d_thi = nc.dram_tensor('thi', [C, M], BF16, kind='ExternalInput').ap()
        d_tlo = nc.dram_tensor('tlo', [C, M], BF16, kind='ExternalInput').ap()
    d_t2row = nc.dram_tensor('t2row', [1, M], F32, kind='ExternalInput').ap()
    d_e2cols = nc.dram_tensor('e2cols', [128, N_TILES], F32, kind='ExternalInput').ap()
    d_ident = nc.dram_tensor('ident', [128, 128], F32, kind='ExternalInput').ap()
    d_z = nc.dram_tensor('z', [B_CORE, M, HW], F32, kind='ExternalOutput').ap()

    with tile.TileContext(nc) as tc:
        with ExitStack() as ctx:
            sbb = cfg['sb_bufs']
            # route px_tiles with t%8 < dve_tr_mod through the DVE
            # stream-transpose path, the rest through PE transposes
            dve_mod = 8 if cfg.get('dve_tr') else int(cfg.get('dve_tr_mod', 0))
            all_dve = dve_mod >= 8
            singles = ctx.enter_context(tc.tile_pool(name='singles', bufs=1))
            encp = ctx.enter_context(tc.tile_pool(name='encp', bufs=4))
            mmps = ctx.enter_context(
                tc.tile_pool(name='mmps', bufs=8 if all_dve else cfg['mm_bufs'],
                             space='PSUM'))
            trps = None if all_dve else ctx.enter_context(
                tc.tile_pool(name='trps', bufs=cfg['tr_bufs'], space='PSUM'))
            bpool = ctx.enter_context(tc.tile_pool(name='bpool', bufs=sbb))
            epool = ctx.enter_context(tc.tile_pool(name='epool', bufs=sbb))
            zpool = ctx.enter_context(tc.tile_pool(name='zpool', bufs=sbb))
            spool = ctx.enter_context(tc.tile_pool(name='spool', bufs=2 * sbb))
            opool = ctx.enter_context(tc.tile_pool(name='opool', bufs=sbb + 1))

            # Persistent tiles: codebook (both bf16 halves, split by k-chunk),
            # broadcast t2, e2 columns, identity.
            mm_dt = F32R if mm_mode == 'f32r' else BF16
            t_thi = [singles.tile([128, M], mm_dt, name=f'thi{k}', tag=f'thi{k}')
                     for k in range(2)]
            for k in range(2):
                eng = nc.sync if k == 0 else nc.scalar
                eng.dma_start(t_thi[k][:], d_thi[k * 128:(k + 1) * 128, :])
            if mm_mode != 'f32r':
                t_tlo = [singles.tile([128, M], BF16, name=f'tlo{k}', tag=f'tlo{k}')
                         for k in range(2)]
                for k in range(2):
                    eng = nc.scalar if k == 0 else nc.sync
                    eng.dma_start(t_tlo[k][:], d_tlo[k * 128:(k + 1) * 128, :])
            t_t2b = singles.tile([128, M], F32)
            nc.scalar.dma_start(t_t2b[:], d_t2row.partition_broadcast(128))
            t_e2c = singles.tile([128, N_TILES], F32)
            nc.sync.dma_start(t_e2c[:, 0:n_tiles], d_e2cols[:, 0:n_tiles])
            t_I = singles.tile([128, 128], F32)
            nc.sync.dma_start(t_I[:], d_ident)

            z_view = d_z.rearrange('b (s p) q -> b p s q', p=128)  # [B_CORE,128,16,1024]
            # 32-block scatter view for the DVE stream-transpose path:
            # dims (b, qc, pi, fb, fi): m = fb*32+pi, px = qc*32+fi
            z_blk = d_z.rearrange('b (fb pi) (qc fi) -> b qc pi fb fi',
                                  pi=32, fi=32)

            for t in range(n_tiles):
                b, pt = divmod(t, HW // 128)
                px0 = pt * 128
                e2col = t_e2c[:, t:t + 1]

                # encode tiles (stationary operands), both k-chunks, both halves
                e_t = {}
                halves = (('h', d_ehi),) if mm_mode == 'f32r' else \
                    (('h', d_ehi), ('l', d_elo))
                load_eng = nc.scalar if t % 2 == 0 else nc.sync
                for k in range(2):
                    for half, src in halves:
                        til = encp.tile([128, 128], mm_dt, name=f'e{half}{k}',
                                        tag=f'e{half}{k}')
                        load_eng.dma_start(
                            til[:], src[b, k * 128:(k + 1) * 128, px0:px0 + 128])
                        e_t[(half, k)] = til

                t_B = bpool.tile([128, M], F32, tag='B')
                pms = [mmps.tile([128, 512], F32, name=f'pm{j}', tag='mm')
                       for j in range(4)]
                # Stationary-major order: one LDWEIGHTS per encode tile.
                if mm_mode == 'f32r':
                    passes = [('h', 0, ['hi'], 'start'), ('h', 1, ['hi'], 'stop')]
                else:
                    # Per-psum accumulation order: eh0*thi0(start), eh0*tlo0,
                    # el0*thi0, eh1*thi1, eh1*tlo1, el1*thi1(stop).
                    passes = [('h', 0, ['hi', 'lo'], 'start'),
                              ('l', 0, ['hi'], None),
                              ('h', 1, ['hi', 'lo'], None),
                              ('l', 1, ['hi'], 'stop')]
                for eh, k, ths, flag in passes:
                    for th in ths:
                        src = (t_thi if th == 'hi' else t_tlo)[k]
                        for j in range(4):
                            js = slice(j * 512, (j + 1) * 512)
                            nc.tensor.matmul(
                                pms[j][:], lhsT=e_t[(eh, k)][:], rhs=src[:, js],
                                start=(flag == 'start' and th == 'hi'),
                                stop=(flag == 'stop'))
                for j in range(4):
                    js = slice(j * 512, (j + 1) * 512)
                    # B = (t2 + e2) - 2*cross   (single f32 rounding per step)
                    nc.vector.scalar_tensor_tensor(
                        out=t_B[:, js], in0=t_t2b[:, js], scalar=e2col, in1=pms[j][:],
                        op0=mybir.AluOpType.add, op1=mybir.AluOpType.subtract)

                # E = exp(e2 - B), S = sum_m E   (fused)
                t_E = epool.tile([128, M], F32, tag='E')
                t_S = spool.tile([128, 1], F32, tag='S')
                nc.scalar.activation(out=t_E[:], in_=t_B[:],
                                     func=mybir.ActivationFunctionType.Exp,
                                     bias=e2col, scale=-1.0, accum_out=t_S[:])
                t_r = spool.tile([128, 1], F32, tag='r')
                nc.vector.reciprocal(t_r[:], t_S[:])

                # Z = E * (1/S)  (per-partition scalar)
                t_Z = zpool.tile([128, M], F32, tag='Z')
                dve_path = (t % 8) < dve_mod
                if dve_path:
                    nc.scalar.mul(t_Z[:], t_E[:], t_r[:])
                else:
                    nc.vector.tensor_scalar_mul(t_Z[:], t_E[:], t_r[:])

                dma_eng = nc.sync if t % 2 == 0 else nc.scalar
                if dve_path:
                    # 32x32 block-local transpose on DVE, then block-scatter DMA
                    t_T = opool.tile([128, M], F32, tag='T')
                    nc.vector.transpose(t_T[:], t_Z[:])
                    for pb in range(4):
                        dma_eng.dma_start(
                            z_blk[b, pt * 4 + pb, :, :, :],
                            t_T[pb * 32:(pb + 1) * 32, :])
                    continue

                # transpose 16 (128,128) blocks via PE; 4 blocks per PSUM bank
                t_O = opool.tile([128, 16, 128], F32, tag='O')
                for q in range(4):
                    trp = trps.tile([128, 512], F32, name='trp', tag='tr')
                    for sl in range(4):
                        s = q * 4 + sl
                        nc.tensor.transpose(trp[:, sl * 128:(sl + 1) * 128],
                                            t_Z[:, s * 128:(s + 1) * 128], t_I[:])
                    # drain PSUM->SBUF (split between ACT and DVE)
                    dst = t_O[:, q * 4:(q + 1) * 4, :].rearrange('p a b -> p (a b)')
                    if q % 2 == 0:
                        nc.scalar.copy(dst, trp[:])
                    else:
                        nc.vector.tensor_copy(dst, trp[:])
                    if cfg.get('dma_per_q'):
                        dma_eng.dma_start(
                            z_view[b, :, 4 * q:4 * (q + 1), px0:px0 + 128],
                            t_O[:, q * 4:(q + 1) * 4, :])

                # alternate output DMAs across the two HW DGE queues (SP / ACT)
                if not cfg.get('dma_per_q'):
                    dma_eng.dma_start(z_view[b, :, :, px0:px0 + 128], t_O[:])

    nc.compile()
    return nc


def _get_nc(n_tiles=N_TILES, mm_mode='bf16split', cfg=None):
    key = (n_tiles, mm_mode, tuple(sorted((cfg or {}).items())))
    if key not in _NC_CACHE:
        _NC_CACHE[key] = build_program(n_tiles, mm_mode, cfg)
    return _NC_CACHE[key]


def _prep_inputs(encode, templat, mm_mode='bf16split'):
    encode = np.ascontiguousarray(encode, dtype=np.float32)
    templat = np.ascontiguousarray(templat, dtype=np.float32)

    e2 = (encode * encode).sum(axis=1, dtype=np.float32).reshape(B_FULL, HW)
    t2 = (templat * templat).sum(axis=1, dtype=np.float32)
    tt2 = np.ascontiguousarray(2.0 * templat.T).astype(np.float32)  # (C, M)

    ef = encode.reshape(B_FULL, C, HW)
    if mm_mode == 'f32r':
        thi, tlo = tt2, None
        ehi, elo = ef, None
    else:
        bft = ml_dtypes.bfloat16
        thi = tt2.astype(bft)
        tlo = (tt2 - thi.astype(np.float32)).astype(bft)
        ehi = ef.astype(bft)
        elo = (ef - ehi.astype(np.float32)).astype(bft)
    ident = np.eye(128, dtype=np.float32)
    t2row = t2.reshape(1, M)

    in_maps = []
    for core in range(N_CORES):
        bs = slice(B_CORE * core, B_CORE * (core + 1))
        e2c = e2[bs].reshape(B_CORE, HW // 128, 128)       # [b, pt, p]
        e2cols = np.ascontiguousarray(e2c.transpose(2, 0, 1).reshape(128, N_TILES))
        m = {'ehi': np.ascontiguousarray(ehi[bs]), 'thi': thi,
             't2row': t2row, 'e2cols': e2cols, 'ident': ident}
        if mm_mode != 'f32r':
            m['elo'] = np.ascontiguousarray(elo[bs])
            m['tlo'] = tlo
        in_maps.append(m)
    return in_maps


def kernel(encode, templat, _trace=False, _n_tiles=N_TILES, _mm_mode='bf16split',
           _cfg=None):
    in_maps = _prep_inputs(encode, templat, _mm_mode)
    nc = _get_nc(_n_tiles, _mm_mode, _cfg)
    res = run_bass_kernel_spmd(nc, in_maps, core_ids=list(range(N_CORES)),
                               trace=_trace)
    zsoft = np.empty((B_FULL, M, H, W), dtype=np.float32)
    for core in range(N_CORES):
        zsoft[B_CORE * core:B_CORE * (core + 1)] = \
            res.results[core]['z'].reshape(B_CORE, M, H, W)
    zidx = np.argmax(zsoft.reshape(B_FULL, M, HW), axis=1).astype(np.int32)
    zidx = zidx.reshape(B_FULL, H, W)
    if _trace:
        kernel.last_exec_time_ns = res.exec_time_ns
        kernel.last_results = res
    return zsoft, zidx


# revision 41
# speedup vs baseline: 1.0136x; 1.0136x over previous
"""VQ codebook kernel (nn_Codebook) for 8 Trainium2 NeuronCores.

Computes, for encode (32,256,32,32) f32 and templat (2048,256) f32:
    dist  = ||e||^2 + ||t||^2 - 2 e.t          (b, m, h, w)
    zsoft = softmax(-dist, axis=m)             (32, 2048, 32, 32) f32
    zidx  = argmax(zsoft, axis=m)              (32, 32, 32) int32

Sharding: data-parallel over batch (4 batches per core), codebook replicated.

Numerical strategy (matters for argmax ties): the reference's dist is
computed in f32 at magnitude ~||e||^2 ~ 256, which quantizes the per-m
variation to a ~1.5e-5 grid. We mimic that arithmetic exactly on device:
  B = fl(fl(e2 + t2) - 2*cross)   with 2*cross from a bf16-split matmul
      (hi*hi + hi*lo + lo*hi, error ~3e-7 << grid)
  zsoft_unnorm = exp(e2 - B)      (ACT, fused bias, exact Sterbenz shift)
  zsoft = zsoft_unnorm * (1/sum)  per pixel
which reproduces the reference's argmax decisions (validated: 0 flips of
32768 vs a CPU-jax reference) while keeping zsoft within ~1e-7 relative.
"""

import sys

if '/opt/trn_rl_repo' not in sys.path:
    sys.path.insert(0, '/opt/trn_rl_repo')

import types
import numpy as np
import ml_dtypes
from contextlib import ExitStack

import concourse.tile as tile
from concourse import bacc, mybir
from concourse.bass_utils import run_bass_kernel_spmd

# bass_utils imports antenv.axon_hooks when tracing is requested (e.g. via a
# stray BASS_TRACE env). This image's antenv lacks that module; register a
# no-op fallback so tracing degrades gracefully instead of ImportError-ing.
if 'antenv.axon_hooks' not in sys.modules:
    _m = types.ModuleType('antenv.axon_hooks')
    _m._hook = None
    _m.set_axon_ntff_profile_hook = lambda h: setattr(_m, '_hook', h)
    _m.get_axon_ntff_profile_hook = lambda: _m._hook
    sys.modules['antenv.axon_hooks'] = _m

B_FULL, C, H, W = 32, 256, 32, 32
M = 2048
HW = H * W
N_CORES = 8
B_CORE = B_FULL // N_CORES          # 4 batches per core
N_TILES = B_CORE * (HW // 128)      # 32 pixel-tiles of 128 per core

F32 = mybir.dt.float32
BF16 = mybir.dt.bfloat16

_NC_CACHE = {}


def build_program(n_tiles=N_TILES, mm_mode='bf16split', cfg=None):
    cfg = dict({'mm_bufs': 6, 'tr_bufs': 2, 'sb_bufs': 2}, **(cfg or {}))
    nc = bacc.Bacc('TRN2', target_bir_lowering=False, debug=False,
                   num_devices=N_CORES)

    F32R = mybir.dt.float32r
    if mm_mode == 'f32r':
        d_ehi = nc.dram_tensor('ehi', [B_CORE, C, HW], F32R, kind='ExternalInput').ap()
        d_thi = nc.dram_tensor('thi', [C, M], F32R, kind='ExternalInput').ap()
        d_elo = d_tlo = None
    else:
        d_ehi = nc.dram_tensor('ehi', [B_CORE, C, HW], BF16, kind='ExternalInput').ap()
        d_elo = nc.dram_tensor('elo', [B_CORE, C, HW], BF16, kind='ExternalInput').ap()
        d_thi = nc.dram_tensor('thi', [C, M], BF16, kind='ExternalInput').ap()
        d_tlo = nc.dram_tensor('tlo', [C, M], BF16, kind='ExternalInput').ap()
    d_t2row = nc.dram_tensor('t2row', [1, M], F32, kind='ExternalInput').ap()
    d_e2cols = nc.dram_tensor('e2cols', [128, N_TILES], F32, kind='ExternalInput').ap()
    d_ident = nc.dram_tensor('ident', [128, 128], F32, kind='ExternalInput').ap()
    d_z = nc.dram_tensor('z', [B_CORE, M, HW], F32, kind='ExternalOutput').ap()

    with tile.TileContext(nc) as tc:
        with ExitStack() as ctx:
            sbb = cfg['sb_bufs']
            # route px_tiles with t%8 < dve_tr_mod through the DVE
            # stream-transpose path, the rest through PE transposes
            dve_mod = 8 if cfg.get('dve_tr') else int(cfg.get('dve_tr_mod', 0))
            all_dve = dve_mod >= 8
            singles = ctx.enter_context(tc.tile_pool(name='singles', bufs=1))
            encp = ctx.enter_context(tc.tile_pool(name='encp', bufs=4))
            mmps = ctx.enter_context(
                tc.tile_pool(name='mmps', bufs=8 if all_dve else cfg['mm_bufs'],
                             space='PSUM'))
            trps = None if all_dve else ctx.enter_context(
                tc.tile_pool(name='trps', bufs=cfg['tr_bufs'], space='PSUM'))
            bpool = ctx.enter_context(tc.tile_pool(name='bpool', bufs=sbb))
            epool = ctx.enter_context(tc.tile_pool(name='epool', bufs=sbb))
            zpool = ctx.enter_context(tc.tile_pool(name='zpool', bufs=sbb))
            spool = ctx.enter_context(tc.tile_pool(name='spool', bufs=2 * sbb))
            opool = ctx.enter_context(tc.tile_pool(name='opool', bufs=sbb + 1))

            # Persistent tiles: codebook (both bf16 halves, split by k-chunk),
            # broadcast t2, e2 columns, identity.
            mm_dt = F32R if mm_mode == 'f32r' else BF16
            t_thi = [singles.tile([128, M], mm_dt, name=f'thi{k}', tag=f'thi{k}')
                     for k in range(2)]
            # startup schedule: order each queue by first-use time; t2b halves
            # ride both queues so tile-0's stt isn't gated on one 1MB transfer
            t_t2b = singles.tile([128, M], F32)
            t_e2c = singles.tile([128, N_TILES], F32)
            t_I = singles.tile([128, 128], F32)
            nc.sync.dma_start(t_thi[0][:], d_thi[0:128, :])
            nc.scalar.dma_start(t_thi[1][:], d_thi[128:256, :])
            nc.sync.dma_start(t_e2c[:, 0:n_tiles], d_e2cols[:, 0:n_tiles])
            nc.sync.dma_start(t_t2b[:, 0:1024],
                              d_t2row[:, 0:1024].partition_broadcast(128))
            nc.scalar.dma_start(t_t2b[:, 1024:2048],
                                d_t2row[:, 1024:2048].partition_broadcast(128))
            if mm_mode != 'f32r':
                t_tlo = [singles.tile([128, M], BF16, name=f'tlo{k}', tag=f'tlo{k}')
                         for k in range(2)]
                nc.scalar.dma_start(t_tlo[0][:], d_tlo[0:128, :])
                nc.sync.dma_start(t_tlo[1][:], d_tlo[128:256, :])
            nc.sync.dma_start(t_I[:], d_ident)

            z_view = d_z.rearrange('b (s p) q -> b p s q', p=128)  # [B_CORE,128,16,1024]
            # 32-block scatter view for the DVE stream-transpose path:
            # dims (b, qc, pi, fb, fi): m = fb*32+pi, px = qc*32+fi
            z_blk = d_z.rearrange('b (fb pi) (qc fi) -> b qc pi fb fi',
                                  pi=32, fi=32)

            for t in range(n_tiles):
                b, pt = divmod(t, HW // 128)
                px0 = pt * 128
                e2col = t_e2c[:, t:t + 1]

                # encode tiles (stationary operands), both k-chunks, both halves
                e_t = {}
                halves = (('h', d_ehi),) if mm_mode == 'f32r' else \
                    (('h', d_ehi), ('l', d_elo))
                load_eng = nc.sync if t % 2 == 0 else nc.scalar
                for k in range(2):
                    for half, src in halves:
                        til = encp.tile([128, 128], mm_dt, name=f'e{half}{k}',
                                        tag=f'e{half}{k}')
                        load_eng.dma_start(
                            til[:], src[b, k * 128:(k + 1) * 128, px0:px0 + 128])
                        e_t[(half, k)] = til

                t_B = bpool.tile([128, M], F32, tag='B')
                pms = [mmps.tile([128, 512], F32, name=f'pm{j}', tag='mm')
                       for j in range(4)]
                # Stationary-major order: one LDWEIGHTS per encode tile.
                if mm_mode == 'f32r':
                    passes = [('h', 0, 'hi', True, False), ('h', 1, 'hi', False, True)]
                else:
                    # All thi products first (tlo codebook chunks arrive later
                    # in the startup DMA schedule), one LDW-run per stationary.
                    passes = [('h', 0, 'hi', True, False),
                              ('l', 0, 'hi', False, False),
                              ('h', 1, 'hi', False, False),
                              ('l', 1, 'hi', False, False),
                              ('h', 0, 'lo', False, False),
                              ('h', 1, 'lo', False, True)]
                for eh, k, th, st, sp in passes:
                    src = (t_thi if th == 'hi' else t_tlo)[k]
                    for j in range(4):
                        js = slice(j * 512, (j + 1) * 512)
                        nc.tensor.matmul(
                            pms[j][:], lhsT=e_t[(eh, k)][:], rhs=src[:, js],
                            start=st, stop=sp)
                for j in range(4):
                    js = slice(j * 512, (j + 1) * 512)
                    # B = (t2 + e2) - 2*cross   (single f32 rounding per step)
                    nc.vector.scalar_tensor_tensor(
                        out=t_B[:, js], in0=t_t2b[:, js], scalar=e2col, in1=pms[j][:],
                        op0=mybir.AluOpType.add, op1=mybir.AluOpType.subtract)

                # E = exp(e2 - B), S = sum_m E   (fused)
                t_E = epool.tile([128, M], F32, tag='E')
                t_S = spool.tile([128, 1], F32, tag='S')
                nc.scalar.activation(out=t_E[:], in_=t_B[:],
                                     func=mybir.ActivationFunctionType.Exp,
                                     bias=e2col, scale=-1.0, accum_out=t_S[:])
                t_r = spool.tile([128, 1], F32, tag='r')
                nc.vector.reciprocal(t_r[:], t_S[:])

                # Z = E * (1/S)  (per-partition scalar)
                t_Z = zpool.tile([128, M], F32, tag='Z')
                dve_path = (t % 8) < dve_mod
                if dve_path:
                    nc.scalar.mul(t_Z[:], t_E[:], t_r[:])
                else:
                    nc.vector.tensor_scalar_mul(t_Z[:], t_E[:], t_r[:])

                dma_eng = nc.sync if t % 2 == 0 else nc.scalar
                if dve_path:
                    # 32x32 block-local transpose on DVE, then block-scatter DMA
                    t_T = opool.tile([128, M], F32, tag='T')
                    nc.vector.transpose(t_T[:], t_Z[:])
                    for pb in range(4):
                        dma_eng.dma_start(
                            z_blk[b, pt * 4 + pb, :, :, :],
                            t_T[pb * 32:(pb + 1) * 32, :])
                    continue

                # transpose 16 (128,128) blocks via PE; 4 blocks per PSUM bank
                t_O = opool.tile([128, 16, 128], F32, tag='O')
                for q in range(4):
                    trp = trps.tile([128, 512], F32, name='trp', tag='tr')
                    for sl in range(4):
                        s = q * 4 + sl
                        nc.tensor.transpose(trp[:, sl * 128:(sl + 1) * 128],
                                            t_Z[:, s * 128:(s + 1) * 128], t_I[:])
                    # drain PSUM->SBUF (split between ACT and DVE)
                    dst = t_O[:, q * 4:(q + 1) * 4, :].rearrange('p a b -> p (a b)')
                    if cfg.get('copies_all_act') or q % 2 == 0:
                        nc.scalar.copy(dst, trp[:])
                    else:
                        nc.vector.tensor_copy(dst, trp[:])
                    if cfg.get('dma_per_q'):
                        dma_eng.dma_start(
                            z_view[b, :, 4 * q:4 * (q + 1), px0:px0 + 128],
                            t_O[:, q * 4:(q + 1) * 4, :])

                # alternate output DMAs across the two HW DGE queues (SP / ACT)
                if not cfg.get('dma_per_q'):
                    dma_eng.dma_start(z_view[b, :, :, px0:px0 + 128], t_O[:])

    nc.compile()
    return nc


def _get_nc(n_tiles=N_TILES, mm_mode='bf16split', cfg=None):
    key = (n_tiles, mm_mode, tuple(sorted((cfg or {}).items())))
    if key not in _NC_CACHE:
        _NC_CACHE[key] = build_program(n_tiles, mm_mode, cfg)
    return _NC_CACHE[key]


def _prep_inputs(encode, templat, mm_mode='bf16split'):
    encode = np.ascontiguousarray(encode, dtype=np.float32)
    templat = np.ascontiguousarray(templat, dtype=np.float32)

    e2 = (encode * encode).sum(axis=1, dtype=np.float32).reshape(B_FULL, HW)
    t2 = (templat * templat).sum(axis=1, dtype=np.float32)
    tt2 = np.ascontiguousarray(2.0 * templat.T).astype(np.float32)  # (C, M)

    ef = encode.reshape(B_FULL, C, HW)
    if mm_mode == 'f32r':
        thi, tlo = tt2, None
        ehi, elo = ef, None
    else:
        bft = ml_dtypes.bfloat16
        thi = tt2.astype(bft)
        tlo = (tt2 - thi.astype(np.float32)).astype(bft)
        ehi = ef.astype(bft)
        elo = (ef - ehi.astype(np.float32)).astype(bft)
    ident = np.eye(128, dtype=np.float32)
    t2row = t2.reshape(1, M)

    in_maps = []
    for core in range(N_CORES):
        bs = slice(B_CORE * core, B_CORE * (core + 1))
        e2c = e2[bs].reshape(B_CORE, HW // 128, 128)       # [b, pt, p]
        e2cols = np.ascontiguousarray(e2c.transpose(2, 0, 1).reshape(128, N_TILES))
        m = {'ehi': np.ascontiguousarray(ehi[bs]), 'thi': thi,
             't2row': t2row, 'e2cols': e2cols, 'ident': ident}
        if mm_mode != 'f32r':
            m['elo'] = np.ascontiguousarray(elo[bs])
            m['tlo'] = tlo
        in_maps.append(m)
    return in_maps


def kernel(encode, templat, _trace=False, _n_tiles=N_TILES, _mm_mode='bf16split',
           _cfg=None):
    in_maps = _prep_inputs(encode, templat, _mm_mode)
    nc = _get_nc(_n_tiles, _mm_mode, _cfg)
    res = run_bass_kernel_spmd(nc, in_maps, core_ids=list(range(N_CORES)),
                               trace=_trace)
    zsoft = np.empty((B_FULL, M, H, W), dtype=np.float32)
    for core in range(N_CORES):
        zsoft[B_CORE * core:B_CORE * (core + 1)] = \
            res.results[core]['z'].reshape(B_CORE, M, H, W)
    zidx = np.argmax(zsoft.reshape(B_FULL, M, HW), axis=1).astype(np.int32)
    zidx = zidx.reshape(B_FULL, H, W)
    if _trace:
        kernel.last_exec_time_ns = res.exec_time_ns
        kernel.last_results = res
    return zsoft, zidx
